# revision 1
# baseline (speedup 1.0000x reference)
"""Sparse-attention kernel for nn_Attention_53558242181469, SPMD across 8 trn2 NeuronCores.

Sharding (per spec hint): the 48 total heads (4 branches x 12 sub-heads) are
split 6-per-core. Each core also gets the matching row-slices of Wq/Wk/WO, so
the three big matmuls are sharded too. Per-head work (scores, softmax+sink,
top-k retrieval, V_net MLP) is fully local; a single all-reduce (psum) after
the branch-partial output projection produces the branch sum, which every core
scales by 1/N_BR into the final mean.

Top-k(12) is computed without sort/gather primitives: 12 rounds of
(row-max, select, knock-out) build the top-k-masked probability matrix, and the
weighted key-sum becomes a plain matmul with the vanilla keys.
"""

import functools

import jax
import jax.numpy as jnp
import numpy as np

D_MODEL, N_HEAD, N_BR = 768, 12, 4
DH = D_MODEL // N_HEAD            # 64
H_TOT = N_BR * N_HEAD             # 48
K_RETR = 12
MLP_SCALE = np.pi / np.sqrt(3.0)
N_CORES = 8
HPC = H_TOT // N_CORES            # 6 heads per core
B, T = 2, 1024

_EPS = np.float32(np.finfo(np.float32).eps)


def _rmsnorm(x):
    return x * jax.lax.rsqrt(jnp.mean(x * x, axis=-1, keepdims=True) + _EPS)


@functools.partial(jax.pmap, axis_name="x",
                   static_broadcasted_argnums=())
def _core_fn(A, X, Wq_w, Wq_b, Wk_w, Wk_b, skew, wedge_bias, sink, v_null,
             fc_w, fc_b, proj_w, proj_b, WO_rows, wob_mean, cos, sin, causal):
    # Q projection for this core's 6 heads: (B,T,384) -> (B,6,T,64)
    q = (A @ Wq_w.T + Wq_b).reshape(B, T, HPC, DH).transpose(0, 2, 1, 3)
    q = _rmsnorm(q)
    # K projection for this core's 6 sub-heads (pre-wedge "vanilla" keys)
    k = (X @ Wk_w.T + Wk_b).reshape(B, T, HPC, DH).transpose(0, 2, 1, 3)
    k_vanilla = k

    # BiasedWedge: x + x @ skew + x * diag_bias  (skew shared, bias per head)
    qw = q + jnp.einsum("bhtd,de->bhte", q, skew) + q * wedge_bias[None, :, None, :]
    kw = k + jnp.einsum("bhtd,de->bhte", k, skew) + k * wedge_bias[None, :, None, :]

    # RoPE (interleaved halves concatenated)
    def rope(x):
        x1, x2 = x[..., 0::2], x[..., 1::2]
        return jnp.concatenate([x1 * cos - x2 * sin, x1 * sin + x2 * cos], axis=-1)

    qr, kr = rope(qw), rope(kw)

    scale = DH ** -0.5
    scores = jnp.einsum("bhtd,bhsd->bhts", qr, kr) * scale      # (B,6,T,T)
    scores = jnp.where(causal, -1e30, scores)

    # softmax over [scores, sink]
    sinks = jnp.broadcast_to(sink.reshape(1, HPC, 1, 1), (B, HPC, T, 1))
    m = jnp.maximum(jnp.max(scores, axis=-1, keepdims=True), sinks)
    e_tok = jnp.exp(scores - m)
    e_sink = jnp.exp(sinks - m)
    denom = jnp.sum(e_tok, axis=-1, keepdims=True) + e_sink
    probs_tok = jnp.where(causal, 0.0, e_tok / denom)
    probs_sink = e_sink / denom                                  # (B,6,T,1)

    # top-12 masked probabilities via iterative knock-out (no sort/gather)
    work = probs_tok
    masked = jnp.zeros_like(work)
    for _ in range(K_RETR):
        mx = jnp.max(work, axis=-1, keepdims=True)
        hit = (work == mx) & (mx > 0)
        masked = jnp.where(hit, work, masked)
        work = jnp.where(hit, -1.0, work)

    marker = (jnp.einsum("bhts,bhsd->bhtd", masked, k_vanilla) + k_vanilla) / (K_RETR + 1)

    # per-token V_net MLP on head_dim
    h = marker @ fc_w.T + fc_b
    h = h * h + 0.75 * h * h * h
    h = _rmsnorm(h)
    h = h * jax.nn.sigmoid(MLP_SCALE * h)
    out_tokens = h @ proj_w.T + proj_b                           # (B,6,T,64)

    context = out_tokens + probs_sink * v_null[None, :, None, :]
    # this core's 6 heads are contiguous sub-heads of ONE branch ->
    # a contiguous 384-column slice of that branch's (B,T,768) context
    ctx = context.transpose(0, 2, 1, 3).reshape(B, T, HPC * DH)

    y_part = ctx @ WO_rows                                       # (B,T,768)
    y = jax.lax.psum(y_part, "x")
    return y / N_BR + wob_mean


_CACHE = {}


def _shard_inputs(A, X, Wq_w, Wq_b, Wk_w, Wk_b, wedge_A, wedge_bias,
                  sink_scalars, v_nulls, fc_w, fc_b, proj_w, proj_b, WO, WO_b):
    skew = (wedge_A - wedge_A.T).astype(np.float32)
    inv_freq = 1.0 / (10000.0 ** (np.arange(0, DH, 2, dtype=np.float32) / DH))
    freqs = np.arange(T, dtype=np.float32)[:, None] * inv_freq[None, :]
    cos, sin = np.cos(freqs).astype(np.float32), np.sin(freqs).astype(np.float32)
    causal = np.triu(np.ones((T, T), bool), 1)
    wob_mean = WO_b.mean(axis=0).astype(np.float32)
    vn = v_nulls.reshape(H_TOT, DH)

    sh = {k: [] for k in ("Wq_w", "Wq_b", "Wk_w", "Wk_b", "wb", "sink", "vn", "WO")}
    for d in range(N_CORES):
        h0 = d * HPC                      # first head on this core
        br = h0 // N_HEAD                 # its branch
        s0 = h0 % N_HEAD                  # first sub-head within branch
        sh["Wq_w"].append(Wq_w[h0 * DH:(h0 + HPC) * DH])
        sh["Wq_b"].append(Wq_b[h0 * DH:(h0 + HPC) * DH])
        sh["Wk_w"].append(Wk_w[s0 * DH:(s0 + HPC) * DH])
        sh["Wk_b"].append(Wk_b[s0 * DH:(s0 + HPC) * DH])
        sh["wb"].append(wedge_bias[h0:h0 + HPC])
        sh["sink"].append(sink_scalars[h0:h0 + HPC])
        sh["vn"].append(vn[h0:h0 + HPC])
        sh["WO"].append(WO[br, s0 * DH:(s0 + HPC) * DH, :])

    def rep(x):
        x = np.asarray(x, np.float32)
        return np.broadcast_to(x, (N_CORES,) + x.shape)

    def stk(key):
        return np.ascontiguousarray(np.stack(sh[key]).astype(np.float32))

    return (rep(A), rep(X), stk("Wq_w"), stk("Wq_b"), stk("Wk_w"), stk("Wk_b"),
            rep(skew), stk("wb"), stk("sink"), stk("vn"),
            rep(fc_w), rep(fc_b), rep(proj_w), rep(proj_b), stk("WO"),
            rep(wob_mean), rep(cos), rep(sin),
            np.broadcast_to(causal, (N_CORES, T, T)))


def kernel(**inputs) -> np.ndarray:
    args = _shard_inputs(**{k: np.asarray(v) for k, v in inputs.items()})
    y = _core_fn(*args)
    return np.asarray(y[0], dtype=np.float32)

